# revision 29
# baseline (speedup 1.0000x reference)
"""Trainium2 Bass kernel for nn_PointTransformerLayer_59674275611307.

Mathematical simplification: in the reference, the attention logits `w` are
broadcast identically across the NSAMPLE axis before the softmax.  Softmax
over identical values is exactly uniform (1/16 each), and the weights sum to
exactly 1, so the grouped weighted sum of values collapses to the values
themselves:

    out = (xv_g * attn).sum(axis=1) == xv == x @ Wv + bv

(verified: rel err ~2e-7 vs the full reference).  Everything else — the q/k
projections, the position MLP, both BN+MLP stacks and the softmax — cancels
out of the output entirely.  The kernel computes the single
(50000,64)@(64,64) matmul + bias, data-parallel over points across 8 cores.

Numeric scheme (avoids the fp32 weight-load wall on the PE): split both x
and Wv into fp16 (hi, scaled-lo) planes:

    x  = hi + 2^-11 * lo_s + O(2^-22)        (lo_s = f16((x - hi) * 2048))
    Wv = Wvb + 2^-11 * Wvr_s + O(2^-22)

    x @ Wv ~= hi @ Wvb  +  2^-11 * (lo_s @ Wvb + hi @ Wvr_s)

Both planes are in fp16 normal range (no subnormals), all products are
exact in the PE's fp32 accumulation, and the dropped lo_s@Wvr_s term is
~2^-22 relative — fp32-level accuracy overall (measured ~5e-7).

Device strategy (per core, 6400 rows after padding 50000 -> 51200):
  - host packs one dram tensor "xkw" [128, 128 + 6400] fp16: a 128-col
    header (Wvb stacked twice; Wvr_s) followed by the x planes transposed:
    partition k in 0:64 = hi[k, row n], partition 64+k = lo_s[k, row n],
    column 128+n.  The contraction dim k is the SBUF partition dim (the PE
    contracts over partitions) with zero on-device transposes and fully
    contiguous 128-partition DMAs.
  - weights are the PE-stationary operand (tiny, reloaded from SBUF in
    ~50ns), data is the moving operand at 1 cycle/row — no per-chunk
    weight-load wall.  Output is produced transposed (out.T[c, n] in PSUM);
    the host untransposes during the gather.
  - 16 batches of 400 rows; batches 2i/2i+1 write the top/bottom 64
    partitions of shared PSUM banks (concurrent via PE column-groups), one
    main + one correction bank per pair; a single DVE op per pair fuses
    (corr * 2^-11 + main) while copying PSUM -> SBUF.
  - dummy bf16 matmuls warm the PE HAM clock gate during the input DMAs.
"""

import numpy as np

N = 50000
C = 64
NCORES = 8
ROWS_PER_CORE = 6400
N_PAD = NCORES * ROWS_PER_CORE  # 51200
BATCH = 400                   # rows per matmul (moving free dim)
NBATCH = ROWS_PER_CORE // BATCH  # 16
NPAIR = NBATCH // 2           # 8 (top/bottom share a psum bank pair)
HDR = 128                     # header cols: Wvb (64) + Wvr_s (64)
LO_SCALE = 2048.0             # 2^11
# payload pairs per load piece (piece 0 also carries the header)
PIECE_PAIRS = [2, 2, 2, 2]
N_WARMUP = 4   # dummy bf16 matmuls start the PE HAM activity clock early

TRACE = False          # test.py sets True to collect an NTFF profile
LAST_RESULT = None     # BassKernelResults of the last run (for test.py)

_cache = {}


def _get_compiled():
    if "nc" in _cache:
        return _cache["nc"]

    import concourse.mybir as mybir
    import concourse.tile as tile
    from concourse import bacc
    from concourse.alu_op_type import AluOpType
    from concourse.bass import MemorySpace

    f32 = mybir.dt.float32
    f16 = mybir.dt.float16
    bf16 = mybir.dt.bfloat16
    nc = bacc.Bacc("TRN2", target_bir_lowering=False, debug=False,
                   num_devices=NCORES)

    xkw_d = nc.dram_tensor("xkw", [128, HDR + ROWS_PER_CORE], f16,
                           kind="ExternalInput")
    out_d = nc.dram_tensor("out", [128, NBATCH // 2 * BATCH], f32,
                           kind="ExternalOutput")

    with tile.TileContext(nc) as tc:
        with (
            tc.tile_pool(name="const", bufs=1) as constp,
            tc.tile_pool(name="xt", bufs=1) as xtp,
            tc.tile_pool(name="outp", bufs=1) as outp,
            tc.tile_pool(name="ps", bufs=3, space=MemorySpace.PSUM) as psp,
        ):
            # PE warmup: dummy bf16 matmuls gated only on a memset start
            # the PE HAM activity clock early; the real stream continues it
            # so the clock gate flips to 8/8 a few us into the stream.
            scr = constp.tile([128, 384], bf16, tag="scr")
            nc.gpsimd.memset(scr[:], 0.0)
            ps_w = psp.tile([128, 512], f32, tag="warm", bufs=1)
            for _ in range(N_WARMUP):
                nc.tensor.matmul(ps_w[:, :384], scr[:, :128], scr[:],
                                 start=True, stop=True)

            # Input DMAs: ALL on the sync HWDGE ring, in consumption order.
            # HWDGE executes FIFO per issuing engine, so piece 0 finishes
            # first (cross-ring round-robin would make every piece finish
            # near the end of the total).  Stores go on the scalar ring so
            # they never queue behind loads.
            xt_tiles = []   # (tile, first_batch, payload_col_offset)
            col = 0
            b0 = 0
            for i, npair in enumerate(PIECE_PAIRS):
                w = npair * 2 * BATCH + (HDR if i == 0 else 0)
                t_ = xtp.tile([128, w], f16, tag=f"xt{i}", name=f"xt_sb{i}")
                nc.sync.dma_start(t_[:], xkw_d.ap()[:, col:col + w])
                xt_tiles.append((t_, b0, HDR if i == 0 else 0))
                col += w
                b0 += npair * 2
            # header col 0:64  rows 0:64   = Wvb * 2^11  (main, up-scaled)
            # header col 64:128 rows 0:64  = Wvr_s       (stacked corr wts)
            #                  rows 64:128 = Wvb
            # PSUM accumulates 2^11*(hi@Wvb + lo@Wvb + hi@Wvr); the DVE
            # copy scales by 2^-11.  All fp16 values stay in normal range.
            wv_main = xt_tiles[0][0][0:64, 0:C]      # [64, 64] f16
            wv_corr = xt_tiles[0][0][:, C:2 * C]     # [128, 64] f16

            def batch_cols(b):
                """SBUF AP [128, BATCH] of batch b's moving data."""
                for t_, first, off in xt_tiles:
                    nb = (t_.shape[1] - off) // BATCH
                    if first <= b < first + nb:
                        lo = off + (b - first) * BATCH
                        return t_[:, lo:lo + BATCH]
                raise AssertionError(b)

            out_sb = outp.tile([128, NPAIR * BATCH], f32, tag="out")

            for i in range(NPAIR):
                ps = psp.tile([128, BATCH], f32, tag="acc", bufs=5)
                for h in (0, 1):            # top / bottom half (col groups)
                    b = 2 * i + h
                    rhs = batch_cols(b)
                    sl = slice(64 * h, 64 * h + 64)
                    # main: 2^11 * hi @ Wvb  (contract over partitions 0:64)
                    nc.tensor.matmul(ps[sl, :], wv_main, rhs[0:64, :],
                                     start=True, stop=False)
                    # corr: 2^11*(hi@Wvr + lo@Wvb), accumulated
                    nc.tensor.matmul(ps[sl, :], wv_corr, rhs,
                                     start=False, stop=True)
                # out = psum * 2^-11 (PSUM -> SBUF, one op per pair,
                # alternating DVE / ACT so neither engine is the tail)
                dst = out_sb[:, i * BATCH:(i + 1) * BATCH]
                if i % 2 == 0:
                    nc.vector.tensor_scalar_mul(dst, ps[:, :], 1.0 / LO_SCALE)
                else:
                    nc.scalar.activation(dst, ps[:, :],
                                         mybir.ActivationFunctionType.Copy,
                                         scale=1.0 / LO_SCALE)

            # stores: one per 2 pairs (800 f32 cols = 400KB), scalar ring
            for s in range(NPAIR // 2):
                lo = s * 2 * BATCH
                nc.scalar.dma_start(out_d.ap()[:, lo:lo + 2 * BATCH],
                                    out_sb[:, lo:lo + 2 * BATCH])

    nc.compile()
    _cache["nc"] = nc
    return nc


def pack_inputs(x, Wv):
    """Build the per-core [128, 6528] fp16 xkw arrays (header + planes)."""
    x_pad = np.zeros((N_PAD, C), np.float32)
    x_pad[:N] = x
    xs = x_pad.reshape(NCORES, ROWS_PER_CORE, C)
    hi = xs.astype(np.float16)
    lo_s = ((xs - hi.astype(np.float32)) * LO_SCALE).astype(np.float16)
    xk = np.concatenate([hi.transpose(0, 2, 1), lo_s.transpose(0, 2, 1)],
                        axis=1)                     # [8, 128, 6400] f16
    Wvb = Wv.astype(np.float16)
    Wvr_s = ((Wv - Wvb.astype(np.float32)) * LO_SCALE).astype(np.float16)
    hdr = np.zeros((NCORES, 128, HDR), np.float16)
    hdr[:, 0:64, 0:C] = (Wvb.astype(np.float32) * LO_SCALE).astype(
        np.float16)   # exact power-of-2 scale
    hdr[:, 0:64, C:2 * C] = Wvr_s
    hdr[:, 64:128, C:2 * C] = Wvb
    return np.ascontiguousarray(np.concatenate([hdr, xk], axis=2))


def unpack_output(res_list, bv):
    """[128, 3200] per-core device outputs -> (N, 64), plus bias."""
    outs = []
    for r in res_list:
        o = r["out"].reshape(2, 64, NPAIR, BATCH)
        # out_dev[h*64+c, i*400+j] = out(row (2i+h)*400+j, chan c)
        outs.append(o.transpose(2, 0, 3, 1).reshape(ROWS_PER_CORE, C))
    out = np.concatenate(outs, axis=0)[:N]
    if np.any(bv):
        out = out + bv[None, :].astype(np.float32)
    return np.ascontiguousarray(out.astype(np.float32))


def kernel(**inputs):
    global LAST_RESULT
    x = np.asarray(inputs["x"], dtype=np.float32)
    Wv = np.asarray(inputs["Wv"], dtype=np.float32)
    bv = np.asarray(inputs["bv"], dtype=np.float32)

    nc = _get_compiled()
    xkw = pack_inputs(x, Wv)

    from concourse.bass_utils import run_bass_kernel_spmd
    in_maps = [{"xkw": xkw[i]} for i in range(NCORES)]
    res = run_bass_kernel_spmd(nc, in_maps, list(range(NCORES)),
                               trace=TRACE)
    LAST_RESULT = res
    return unpack_output(res.results, bv)


# revision 30
# speedup vs baseline: 1.1979x; 1.1979x over previous
"""Trainium2 Bass kernel for nn_PointTransformerLayer_59674275611307.

Mathematical simplification: in the reference, the attention logits `w` are
broadcast identically across the NSAMPLE axis before the softmax.  Softmax
over identical values is exactly uniform (1/16 each), and the weights sum to
exactly 1, so the grouped weighted sum of values collapses to the values
themselves:

    out = (xv_g * attn).sum(axis=1) == xv == x @ Wv + bv

(verified: rel err ~2e-7 vs the full reference).  Everything else — the q/k
projections, the position MLP, both BN+MLP stacks and the softmax — cancels
out of the output entirely.  The kernel therefore computes the single
(50000,64)@(64,64) matmul + bias, data-parallel over points across 8 cores.

Device strategy (per core, 6400 rows after padding 50000 -> 51200):
  - host packs the core's shard into ONE dram tensor "xtw" [128, 3328]:
    cols 0:64   = Wv stacked twice on the partition dim (for the two PE
                  row-groups), cols 64:128 = bias replicated, cols 128:3328
    = x transposed in chunk-pairs: partition 64*(t&1)+k, column
    128 + (t>>1)*128 + p holds x[p*50 + t, k]  (t = 128-row chunk index,
    p = row-within-chunk).  The contraction dim k becomes the SBUF
    partition dim (the PE contracts over partitions) with zero on-device
    transposes, full-128-partition DMAs, and contiguous descriptors.
  - 50 fp32 matmuls (lhsT = x-chunk.T [64,128] stationary, rhs = Wv [64,64]
    moving), row-packed in even/odd pairs at tile_position (0,0)/(64,0) so
    consecutive chunks run concurrently in disjoint PE row-groups writing
    separate PSUM banks.
  - dummy bf16 matmuls warm the PE HAM clock gate during the input DMAs.
  - DVE adds the bias while copying PSUM -> SBUF; contiguous stores.
"""

import numpy as np

N = 50000
C = 64
NCORES = 8
T = 50                        # 128-row chunks per core
ROWS_PER_CORE = 128 * T       # 6400
N_PAD = NCORES * ROWS_PER_CORE  # 51200
PAIRS = T // 2                # 25
XT_COLS = PAIRS * 128         # 3200
HDR = 128                     # wv (64) + bias (64) header columns
GROUP = 8                     # chunks per psum group (split 4 even + 4 odd)
# xt load pieces in pairs-of-chunks; piece 0 additionally carries the header
XT_PIECE_PAIRS = [4, 7, 7, 7]
# Output store pieces (columns of the [128, 3200] out-sbuf layout).
OUT_PIECES = [(0, 1024), (1024, 2048), (2048, 3072), (3072, 3200)]
N_WARMUP = 12  # dummy bf16 matmuls to engage the PE HAM clock during DMA-in

TRACE = False          # test.py sets True to collect an NTFF profile
LAST_RESULT = None     # BassKernelResults of the last run (for test.py)

_cache = {}


def _get_compiled():
    if "nc" in _cache:
        return _cache["nc"]

    import concourse.mybir as mybir
    import concourse.tile as tile
    from concourse import bacc
    from concourse.bass import MemorySpace

    f32 = mybir.dt.float32
    bf16 = mybir.dt.bfloat16
    nc = bacc.Bacc("TRN2", target_bir_lowering=False, debug=False,
                   num_devices=NCORES)

    xtw_d = nc.dram_tensor("xtw", [128, HDR + XT_COLS], f32,
                           kind="ExternalInput")
    out_d = nc.dram_tensor("out", [ROWS_PER_CORE, C], f32,
                           kind="ExternalOutput")

    # out DRAM viewed as [partition p, (chunk t, channel k)]: row p*T + t
    out_pt = out_d.ap().rearrange("(p t) k -> p (t k)", p=128)

    with tile.TileContext(nc) as tc:
        with (
            tc.tile_pool(name="const", bufs=1) as constp,
            tc.tile_pool(name="xt", bufs=1) as xtp,
            tc.tile_pool(name="outp", bufs=1) as outp,
            tc.tile_pool(name="ps", bufs=3, space=MemorySpace.PSUM) as psp,
        ):
            # PE warmup: dummy bf16 matmuls gated only on a DVE memset, so
            # they run during the input-DMA window and the HAM clock gate
            # reaches 8/8 before the real (fp32) matmul stream starts.
            scr = constp.tile([128, 384], bf16, tag="scr")
            nc.gpsimd.memset(scr[:], 0.0)
            ps_w = psp.tile([128, 512], f32, tag="warm", bufs=1)
            for _ in range(N_WARMUP):
                nc.tensor.matmul(ps_w[:, :384], scr[:, :128], scr[:],
                                 start=True, stop=True)

            # Input DMAs: piece 0 (header + first pairs) first on sync;
            # remaining pieces alternate sync/scalar so HWDGE descriptor
            # generation and ring drain pipeline across both rings.
            xt_tiles = []   # (tile, first_pair, col_off_of_first_pair)
            col = 0
            for i, npair in enumerate(XT_PIECE_PAIRS):
                w = npair * 128 + (HDR if i == 0 else 0)
                t_ = xtp.tile([128, w], f32, tag=f"xt{i}", name=f"xt_sb{i}")
                nc.sync.dma_start(t_[:], xtw_d.ap()[:, col:col + w])
                first_pair = 0 if i == 0 else (col - HDR) // 128
                xt_tiles.append((t_, first_pair, HDR if i == 0 else 0))
                col += w
            wv = xt_tiles[0][0][:, 0:C]
            bias = xt_tiles[0][0][:, C:2 * C]

            def lhsT_of(p2, a):
                for t_, first, off in xt_tiles:
                    npair = (t_.shape[1] - off) // 128
                    if first <= p2 < first + npair:
                        local = off + (p2 - first) * 128
                        return t_[64 * a:64 * (a + 1), local:local + 128]
                raise AssertionError(p2)

            out_tiles = []
            for i, (lo, hi) in enumerate(OUT_PIECES):
                out_tiles.append(
                    outp.tile([128, hi - lo], f32, tag=f"out{i}",
                              name=f"out_sb{i}"))

            def out_piece_of(col):
                for i, (lo, hi) in enumerate(OUT_PIECES):
                    if lo <= col < hi:
                        return i, col - lo
                raise AssertionError(col)

            n_groups = (T + GROUP - 1) // GROUP
            for g in range(n_groups):
                t0 = g * GROUP
                t1 = min(t0 + GROUP, T)
                nhalf = (t1 - t0) // 2          # chunks per parity
                ps_e = psp.tile([128, 256], f32, tag="mme")
                ps_o = psp.tile([128, 256], f32, tag="mmo")
                for t in range(t0, t1):
                    a = t & 1
                    p2 = t >> 1
                    lhsT = lhsT_of(p2, a)
                    rhs = wv[64 * a:64 * (a + 1), :]
                    j = (t - t0) >> 1
                    ps = ps_e if a == 0 else ps_o
                    nc.tensor.matmul(ps[:, j * 64:(j + 1) * 64], lhsT, rhs,
                                     start=True, stop=True)

                # bias-add PSUM -> out sbuf (even chunks then odd chunks).
                # Out cols for chunk t0+2j+a are (t0+2j+a)*64 — view the
                # group's columns at pair (128-col) granularity, then slice
                # the even/odd 64-col half of each pair.
                opi, ocol = out_piece_of(t0 * 64)
                ot = out_tiles[opi]
                width = nhalf * 64
                ot_pairs = ot[:, ocol:ocol + nhalf * 128].rearrange(
                    "p (j w) -> p j w", w=128)
                bsrc = bias.unsqueeze(1).broadcast_to([128, nhalf, 64])
                for a, ps in ((0, ps_e), (1, ps_o)):
                    dst = ot_pairs[:, :, a * 64:(a + 1) * 64]
                    src = ps[:, :width].rearrange("p (j k) -> p j k", k=64)
                    nc.vector.tensor_add(dst, src, bsrc)

            for i, (lo, hi) in enumerate(OUT_PIECES):
                nc.scalar.dma_start(out_pt[:, lo:hi], out_tiles[i][:])

    nc.compile()
    _cache["nc"] = nc
    return nc


def pack_inputs(x, Wv, bv):
    """Build the per-core [128, 3328] xtw arrays (header + packed x)."""
    x_pad = np.zeros((N_PAD, C), np.float32)
    x_pad[:N] = x
    # xt[core, 64*a + k, p2*128 + p] = x_pad[core*6400 + p*50 + (2*p2+a), k]
    xc = x_pad.reshape(NCORES, 128, PAIRS, 2, C)
    xt = np.ascontiguousarray(xc.transpose(0, 3, 4, 2, 1)).reshape(
        NCORES, 128, XT_COLS)
    xtw = np.empty((NCORES, 128, HDR + XT_COLS), np.float32)
    xtw[:, :64, 0:C] = Wv
    xtw[:, 64:, 0:C] = Wv
    xtw[:, :, C:2 * C] = bv
    xtw[:, :, HDR:] = xt
    return xtw


def kernel(**inputs):
    global LAST_RESULT
    x = np.asarray(inputs["x"], dtype=np.float32)
    Wv = np.asarray(inputs["Wv"], dtype=np.float32)
    bv = np.asarray(inputs["bv"], dtype=np.float32)

    nc = _get_compiled()
    xtw = pack_inputs(x, Wv, bv)

    from concourse.bass_utils import run_bass_kernel_spmd
    in_maps = [{"xtw": xtw[i]} for i in range(NCORES)]
    res = run_bass_kernel_spmd(nc, in_maps, list(range(NCORES)),
                               trace=TRACE)
    LAST_RESULT = res
    out = np.concatenate([res.results[i]["out"] for i in range(NCORES)],
                         axis=0)[:N]
    return np.ascontiguousarray(out)


# revision 34
# speedup vs baseline: 1.3676x; 1.1417x over previous
"""Trainium2 Bass kernel for nn_PointTransformerLayer_59674275611307.

Mathematical simplification: in the reference, the attention logits `w` are
broadcast identically across the NSAMPLE axis before the softmax.  Softmax
over identical values is exactly uniform (1/16 each), and the weights sum to
exactly 1, so the grouped weighted sum of values collapses to the values
themselves:

    out = (xv_g * attn).sum(axis=1) == xv == x @ Wv + bv

(verified: rel err ~2e-7 vs the full reference).  Everything else — the q/k
projections, the position MLP, both BN+MLP stacks and the softmax — cancels
out of the output entirely.  The kernel therefore computes the single
(50000,64)@(64,64) matmul + bias, data-parallel over points across 8 cores.

Device strategy (per core, 6400 rows after padding 50000 -> 51200):
  - host packs the core's shard into ONE dram tensor "xtw" [128, 3328]:
    cols 0:64   = Wv stacked twice on the partition dim (for the two PE
                  row-groups), cols 64:128 = bias replicated, cols 128:3328
    = x transposed in chunk-pairs: partition 64*(t&1)+k, column
    128 + (t>>1)*128 + p holds x[p*50 + t, k]  (t = 128-row chunk index,
    p = row-within-chunk).  The contraction dim k becomes the SBUF
    partition dim (the PE contracts over partitions) with zero on-device
    transposes, full-128-partition DMAs, and contiguous descriptors.
  - 50 fp32 matmuls (lhsT = x-chunk.T [64,128] stationary, rhs = Wv [64,64]
    moving), row-packed in even/odd pairs at tile_position (0,0)/(64,0) so
    consecutive chunks run concurrently in disjoint PE row-groups writing
    separate PSUM banks.
  - dummy bf16 matmuls warm the PE HAM clock gate during the input DMAs.
  - DVE adds the bias while copying PSUM -> SBUF; contiguous stores.
"""

import numpy as np

N = 50000
C = 64
NCORES = 8
T = 50                        # 128-row chunks per core
ROWS_PER_CORE = 128 * T       # 6400
N_PAD = NCORES * ROWS_PER_CORE  # 51200
PAIRS = T // 2                # 25
XT_COLS = PAIRS * 128         # 3200
HDR = 128                     # wv (64) + bias (64) header columns
GROUP = 8                     # chunks per psum group (split 4 even + 4 odd)
# xt load pieces in pairs-of-chunks; piece 0 additionally carries the header
XT_PIECE_PAIRS = [4, 7, 7, 7]
# Output store pieces (columns of the [128, 3200] out-sbuf layout).
OUT_PIECES = [(0, 1024), (1024, 2048), (2048, 3072), (3072, 3200)]
N_WARMUP = 12  # dummy bf16 matmuls to engage the PE HAM clock during DMA-in

TRACE = False          # test.py sets True to collect an NTFF profile
LAST_RESULT = None     # BassKernelResults of the last run (for test.py)

_cache = {}


def _get_compiled():
    if "nc" in _cache:
        return _cache["nc"]

    import concourse.mybir as mybir
    import concourse.tile as tile
    from concourse import bacc
    from concourse.bass import MemorySpace

    f32 = mybir.dt.float32
    bf16 = mybir.dt.bfloat16
    nc = bacc.Bacc("TRN2", target_bir_lowering=False, debug=False,
                   num_devices=NCORES)

    xtw_d = nc.dram_tensor("xtw", [128, HDR + XT_COLS], f32,
                           kind="ExternalInput")
    # Output stored group-contiguous: store s occupies a fully contiguous
    # DRAM region [128, w_s] (HBM writes at 12.8KB partition stride measured
    # only ~190GB/s; contiguous pieces avoid that).  Host reassembles.
    out_d = nc.dram_tensor("out", [128 * XT_COLS], f32,
                           kind="ExternalOutput")

    with tile.TileContext(nc) as tc:
        with (
            tc.tile_pool(name="const", bufs=1) as constp,
            tc.tile_pool(name="xt", bufs=1) as xtp,
            tc.tile_pool(name="outp", bufs=1) as outp,
            tc.tile_pool(name="ps", bufs=3, space=MemorySpace.PSUM) as psp,
        ):
            # PE warmup: dummy bf16 matmuls gated only on a DVE memset, so
            # they run during the input-DMA window and the HAM clock gate
            # reaches 8/8 before the real (fp32) matmul stream starts.
            scr = constp.tile([128, 384], bf16, tag="scr")
            nc.gpsimd.memset(scr[:], 0.0)
            ps_w = psp.tile([128, 512], f32, tag="warm", bufs=1)
            for _ in range(N_WARMUP):
                nc.tensor.matmul(ps_w[:, :384], scr[:, :128], scr[:],
                                 start=True, stop=True)

            # Input DMAs: piece 0 (header + first pairs) first on sync;
            # remaining pieces alternate sync/scalar so HWDGE descriptor
            # generation and ring drain pipeline across both rings.
            xt_tiles = []   # (tile, first_pair, col_off_of_first_pair)
            col = 0
            for i, npair in enumerate(XT_PIECE_PAIRS):
                w = npair * 128 + (HDR if i == 0 else 0)
                t_ = xtp.tile([128, w], f32, tag=f"xt{i}", name=f"xt_sb{i}")
                nc.sync.dma_start(t_[:], xtw_d.ap()[:, col:col + w])
                first_pair = 0 if i == 0 else (col - HDR) // 128
                xt_tiles.append((t_, first_pair, HDR if i == 0 else 0))
                col += w
            wv = xt_tiles[0][0][:, 0:C]
            bias = xt_tiles[0][0][:, C:2 * C]

            def lhsT_of(p2, a):
                for t_, first, off in xt_tiles:
                    npair = (t_.shape[1] - off) // 128
                    if first <= p2 < first + npair:
                        local = off + (p2 - first) * 128
                        return t_[64 * a:64 * (a + 1), local:local + 128]
                raise AssertionError(p2)

            out_tiles = []
            for i, (lo, hi) in enumerate(OUT_PIECES):
                out_tiles.append(
                    outp.tile([128, hi - lo], f32, tag=f"out{i}",
                              name=f"out_sb{i}"))

            def out_piece_of(col):
                for i, (lo, hi) in enumerate(OUT_PIECES):
                    if lo <= col < hi:
                        return i, col - lo
                raise AssertionError(col)

            n_groups = (T + GROUP - 1) // GROUP
            for g in range(n_groups):
                t0 = g * GROUP
                t1 = min(t0 + GROUP, T)
                nhalf = (t1 - t0) // 2          # chunks per parity
                ps_e = psp.tile([128, 256], f32, tag="mme")
                ps_o = psp.tile([128, 256], f32, tag="mmo")
                for t in range(t0, t1):
                    a = t & 1
                    p2 = t >> 1
                    lhsT = lhsT_of(p2, a)
                    rhs = wv[64 * a:64 * (a + 1), :]
                    j = (t - t0) >> 1
                    ps = ps_e if a == 0 else ps_o
                    nc.tensor.matmul(ps[:, j * 64:(j + 1) * 64], lhsT, rhs,
                                     start=True, stop=True)

                # bias-add PSUM -> out sbuf (even chunks then odd chunks).
                # Out cols for chunk t0+2j+a are (t0+2j+a)*64 — view the
                # group's columns at pair (128-col) granularity, then slice
                # the even/odd 64-col half of each pair.
                opi, ocol = out_piece_of(t0 * 64)
                ot = out_tiles[opi]
                width = nhalf * 64
                ot_pairs = ot[:, ocol:ocol + nhalf * 128].rearrange(
                    "p (j w) -> p j w", w=128)
                bsrc = bias.unsqueeze(1).broadcast_to([128, nhalf, 64])
                for a, ps in ((0, ps_e), (1, ps_o)):
                    dst = ot_pairs[:, :, a * 64:(a + 1) * 64]
                    src = ps[:, :width].rearrange("p (j k) -> p j k", k=64)
                    nc.vector.tensor_add(dst, src, bsrc)

            # one store per psum-group (512 out-sbuf cols = 256KB), issued
            # on the scalar ring as soon as that group's bias-adds land
            for g in range(n_groups):
                lo = g * GROUP * C
                hi = min(lo + GROUP * C, XT_COLS)
                w = hi - lo
                opi, ocol = out_piece_of(lo)
                dst = out_d.ap()[128 * lo:128 * hi].rearrange(
                    "(p w) -> p w", p=128)
                nc.scalar.dma_start(dst, out_tiles[opi][:, ocol:ocol + w])

    nc.compile()
    _cache["nc"] = nc
    return nc


def pack_inputs(x, Wv, bv):
    """Build the per-core [128, 3328] xtw arrays (header + packed x)."""
    x_pad = np.zeros((N_PAD, C), np.float32)
    x_pad[:N] = x
    # xt[core, 64*a + k, p2*128 + p] = x_pad[core*6400 + p*50 + (2*p2+a), k]
    xc = x_pad.reshape(NCORES, 128, PAIRS, 2, C)
    xt = np.ascontiguousarray(xc.transpose(0, 3, 4, 2, 1)).reshape(
        NCORES, 128, XT_COLS)
    xtw = np.empty((NCORES, 128, HDR + XT_COLS), np.float32)
    xtw[:, :64, 0:C] = Wv
    xtw[:, 64:, 0:C] = Wv
    xtw[:, :, C:2 * C] = bv
    xtw[:, :, HDR:] = xt
    return xtw


def kernel(**inputs):
    global LAST_RESULT
    x = np.asarray(inputs["x"], dtype=np.float32)
    Wv = np.asarray(inputs["Wv"], dtype=np.float32)
    bv = np.asarray(inputs["bv"], dtype=np.float32)

    nc = _get_compiled()
    xtw = pack_inputs(x, Wv, bv)

    from concourse.bass_utils import run_bass_kernel_spmd
    in_maps = [{"xtw": xtw[i]} for i in range(NCORES)]
    res = run_bass_kernel_spmd(nc, in_maps, list(range(NCORES)),
                               trace=TRACE)
    LAST_RESULT = res
    out = np.concatenate(
        [unpack_core(res.results[i]["out"]) for i in range(NCORES)],
        axis=0)[:N]
    return np.ascontiguousarray(out)


def unpack_core(flat):
    """Flat group-contiguous device output -> (6400, 64) rows."""
    out_sb = flat.reshape(128, XT_COLS)   # group blocks are [128, w] each
    # blocks are stored [128, w] p-major back to back; reassemble columns
    blocks = []
    for lo in range(0, XT_COLS, GROUP * C):
        hi = min(lo + GROUP * C, XT_COLS)
        blocks.append(flat[128 * lo:128 * hi].reshape(128, hi - lo))
    out_sb = np.concatenate(blocks, axis=1)       # [128, (t k)]
    return out_sb.reshape(ROWS_PER_CORE, C)       # row = p*T + t
